# revision 4
# baseline (speedup 1.0000x reference)
"""KPConv-style GNN message passing on 8 TRN2 NeuronCores, v4.

v4 halves the Q7 gather work vs v3: the feature table packs bf16 feature
PAIRS into u32 words (stored as f32 bits): partition p holds the word
(feat 2(p%16), feat 2(p%16)+1), so 16 partitions -- one Q7 core -- cover
a full 32-feature row. Each core gathers its own EIGHTH of the window's
slots (384 idxs vs 640 with pair-duplicated quarters). The per-window PE
transpose runs at u32 granularity (f32 transpose mode), which keeps each
packed word intact: transposed partition rows are whole slots and the
bf16 view of a 32-column block is the natural feature order. Downstream
(fk4 expansion, one scatter matmul per 128-edge chunk into G^T, per-
window weight application) is unchanged from v3.
"""

import numpy as np
import ml_dtypes

E_TOT = 400000
M_NODES = 25000
FI = 32          # input features
FO = 32          # output features
KPTS = 15
NCORES = 8
M_CORE = 3125    # nodes per core
WIN_NODES = 125  # nodes per window
N_WIN = 25       # windows per core
NGRP = 4         # k-groups of 4 kernel points
PAD_COL = 126    # one-hot column for padding edges (row never stored)
NEIGHTH = 8      # ap_gather cores; slot space split in 8 eighths

_CACHE = {}


def _build_nc(layout):
    """layout = tuple of chunks per k-group; sum % 8 == 0."""
    from concourse import bacc, mybir, tile
    from concourse import library_config

    f32 = mybir.dt.float32
    bf16 = mybir.dt.bfloat16
    i16 = mybir.dt.int16
    mult = mybir.AluOpType.mult
    act_copy = mybir.ActivationFunctionType.Copy

    n_ch = sum(layout)
    assert n_ch % NEIGHTH == 0
    jbs = n_ch // NEIGHTH         # transpose blocks (u32 chunks per eighth)
    e_win = n_ch * 128
    eslots = jbs * 128            # slots per eighth per window
    idx_cols = eslots // 16       # idx columns per window in gidx

    grp_of = []
    for g, s in enumerate(layout):
        grp_of += [g] * s
    first = {}
    last = {}
    for c, g in enumerate(grp_of):
        if g not in first:
            first[g] = c
        last[g] = c

    nc = bacc.Bacc("TRN2", target_bir_lowering=False, debug=False)

    featP = nc.declare_dram_parameter("featP", [128, M_NODES], f32, isOutput=False)
    gidx = nc.declare_dram_parameter("gidx", [128, N_WIN * idx_cols], i16, isOutput=False)
    ohw = nc.declare_dram_parameter(
        "ohw", [N_WIN * 128, n_ch * WIN_NODES], bf16, isOutput=False)
    m4 = nc.declare_dram_parameter("m4", [N_WIN * 128, n_ch * 4], bf16, isOutput=False)
    w2g = nc.declare_dram_parameter("w2g", [128, NGRP * FO], bf16, isOutput=False)
    ident = nc.declare_dram_parameter("ident", [128, 128], f32, isOutput=False)
    out = nc.declare_dram_parameter("out", [N_WIN * WIN_NODES, FO], f32, isOutput=True)

    with tile.TileContext(nc) as tc:
        with (
            tc.tile_pool(name="const", bufs=1) as cpool,
            tc.tile_pool(name="win", bufs=2) as wpool,
            tc.tile_pool(name="chunk", bufs=2) as kpool,
            tc.tile_pool(name="gts", bufs=2) as gpool,
            tc.tile_pool(name="ps", bufs=2, space="PSUM") as ppool,
            tc.tile_pool(name="pst", bufs=2, space="PSUM") as tppool,
            tc.tile_pool(name="pso", bufs=2, space="PSUM") as opool,
        ):
            with tc.tile_critical():
                nc.gpsimd.load_library(library_config.ap_gather)

            w2g_sb = cpool.tile([128, NGRP * FO], bf16, tag="w2g")
            ident_sb = cpool.tile([128, 128], f32, tag="ident")
            gidx_sb = cpool.tile([128, N_WIN * idx_cols], i16, tag="gidx")
            featP_sb = cpool.tile([128, M_NODES], f32, tag="featP")
            nc.sync.dma_start(w2g_sb[:], w2g[:])
            nc.sync.dma_start(ident_sb[:], ident[:])
            nc.sync.dma_start(gidx_sb[:], gidx[:])
            nc.sync.dma_start(featP_sb[:], featP[:])

            for w in range(N_WIN):
                ftile = wpool.tile([128, eslots], f32, tag="ftile")
                ohtile = wpool.tile([128, n_ch * WIN_NODES], bf16, tag="ohtile")
                mtile = wpool.tile([128, n_ch * 4], bf16, tag="mtile")

                nc.gpsimd.ap_gather(
                    ftile[:],
                    featP_sb[:],
                    gidx_sb[:, w * idx_cols:(w + 1) * idx_cols],
                    128,          # channels
                    M_NODES,      # num_elems
                    1,            # d
                    eslots,       # num_idxs (per core)
                )
                nc.sync.dma_start(ohtile[:], ohw[w * 128:(w + 1) * 128, :])
                nc.sync.dma_start(mtile[:], m4[w * 128:(w + 1) * 128, :])

                # u32-granular transpose: slot rows, packed words intact
                ps_t = tppool.tile([128, eslots], f32, tag="ps_t")
                for jb in range(jbs):
                    nc.tensor.transpose(
                        ps_t[:, jb * 128:(jb + 1) * 128],
                        ftile[:, jb * 128:(jb + 1) * 128], ident_sb[:])
                tsb = kpool.tile([128, eslots], f32, tag="tsb")
                nc.scalar.activation(tsb[:], ps_t[:], act_copy)

                # fk4 for all 8 eighths of one block in a single DVE op;
                # m4 is laid out jb-major on host: [p, (jb e f)]
                fk4s = []
                for jb in range(jbs):
                    fk4 = kpool.tile([128, NEIGHTH * 128], bf16, tag=f"fk4_{jb}")
                    nc.vector.tensor_tensor(
                        out=fk4[:].rearrange(
                            "p (e f i) -> p e f i", e=NEIGHTH, f=4),
                        in0=tsb[:, jb * 128:(jb + 1) * 128].bitcast(bf16).rearrange(
                            "p (e a i) -> p e a i", e=NEIGHTH, a=1
                        ).broadcast_to([128, NEIGHTH, 4, FI]),
                        in1=mtile[:, jb * 32:(jb + 1) * 32].rearrange(
                            "p (e f a) -> p e f a", e=NEIGHTH, a=1
                        ).broadcast_to([128, NEIGHTH, 4, FI]),
                        op=mult,
                    )
                    fk4s.append(fk4)

                ps_g = ppool.tile([128, NGRP * WIN_NODES], f32, tag="ps_g")
                for c in range(n_ch):
                    e8, jb = c // jbs, c % jbs
                    g = grp_of[c]
                    nc.tensor.matmul(
                        ps_g[:, g * WIN_NODES:(g + 1) * WIN_NODES],
                        fk4s[jb][:, e8 * 128:(e8 + 1) * 128],
                        ohtile[:, c * WIN_NODES:(c + 1) * WIN_NODES],
                        start=(c == first[g]), stop=(c == last[g]),
                    )

                gts = gpool.tile([128, NGRP * WIN_NODES], bf16, tag="gts")
                nc.scalar.activation(gts[:], ps_g[:], act_copy)

                ps_o = opool.tile([128, FO], f32, tag="ps_o")
                for g in range(NGRP):
                    nc.tensor.matmul(
                        ps_o[0:WIN_NODES, :],
                        gts[:, g * WIN_NODES:(g + 1) * WIN_NODES],
                        w2g_sb[:, g * FO:(g + 1) * FO],
                        start=(g == 0), stop=(g == NGRP - 1),
                    )
                osb = kpool.tile([128, FO], f32, tag="osb")
                nc.scalar.activation(osb[0:WIN_NODES, :], ps_o[0:WIN_NODES, :], act_copy)
                nc.sync.dma_start(
                    out[w * WIN_NODES:(w + 1) * WIN_NODES, :], osb[0:WIN_NODES, :])

    nc.compile()
    return nc


def _nearest_k(hood_coords, mu):
    h = hood_coords.astype(np.float32)
    m = mu[0].astype(np.float32)
    d = h[:, None, :] - m[None, :, :]
    return np.einsum('ekc,ekc->ek', d, d).argmin(1)


def _pick_layout(target, k):
    tgt = target.astype(np.int64)
    core_of = tgt // M_CORE
    win_of = (tgt % M_CORE) // WIN_NODES
    grp_of = k // 4
    cnt = np.zeros((NCORES, N_WIN, NGRP), np.int64)
    np.add.at(cnt, (core_of, win_of, grp_of), 1)
    need = np.ceil(cnt.max(axis=(0, 1)) / 128).astype(int)
    need = np.maximum(need, 1)
    # total chunks must split evenly into 8 eighths
    i = 0
    while need.sum() % NEIGHTH:
        need[i % NGRP] += 1
        i += 1
    return tuple(int(x) for x in need)


def _host_prep(source, target, features, hood_coords, mu, w, layout):
    bf = ml_dtypes.bfloat16
    src = np.ascontiguousarray(source.astype(np.int64))
    tgt = np.ascontiguousarray(target.astype(np.int64))
    k = _CACHE["k"]

    n_ch = sum(layout)
    jbs = n_ch // NEIGHTH
    e_win = n_ch * 128
    eslots = jbs * 128
    idx_cols = eslots // 16
    e_pad = N_WIN * e_win
    chunk_base = np.concatenate([[0], np.cumsum(layout)])

    # packed table: partition p holds word (feat 2(p%16), feat 2(p%16)+1)
    fb = np.ascontiguousarray(features.astype(bf))          # [M, 32]
    words = fb.view(np.uint32).reshape(M_NODES, 16)          # [M, 16] pairs
    featP = np.empty((128, M_NODES), dtype=np.uint32)
    for p in range(128):
        featP[p, :] = words[:, p % 16]
    featP = featP.view(np.float32)

    w2 = np.zeros((128, NGRP, FO), dtype=np.float32)
    for g in range(NGRP):
        for krel in range(4):
            kk = 4 * g + krel
            if kk < KPTS:
                w2[krel * FI:(krel + 1) * FI, g, :] = w[:, kk, :].T
    w2g = np.ascontiguousarray(w2.reshape(128, NGRP * FO).astype(bf))

    ident = np.eye(128, dtype=np.float32)

    core_of = tgt // M_CORE
    local = tgt - core_of * M_CORE
    win_of = local // WIN_NODES
    col_of = local - win_of * WIN_NODES
    grp_of_e = k // 4
    krel_of = k - grp_of_e * 4

    bucket = (core_of * N_WIN + win_of) * NGRP + grp_of_e
    order = np.argsort(bucket, kind="stable")
    bounds = np.searchsorted(bucket[order], np.arange(NCORES * N_WIN * NGRP + 1))

    in_maps = []
    for cid in range(NCORES):
        src_p = np.zeros(e_pad, dtype=np.int64)
        col_p = np.full(e_pad, PAD_COL, dtype=np.float32)
        krel_p = np.full(e_pad, -1, dtype=np.int64)
        for wi in range(N_WIN):
            for g in range(NGRP):
                b = (cid * N_WIN + wi) * NGRP + g
                sel = order[bounds[b]:bounds[b + 1]]
                n = len(sel)
                cap = layout[g] * 128
                if n > cap:
                    raise RuntimeError(f"group overflow: {n} > {cap}")
                base = wi * e_win + chunk_base[g] * 128
                src_p[base:base + n] = src[sel]
                col_p[base:base + n] = col_of[sel]
                krel_p[base:base + n] = krel_of[sel]

        # gidx: per-core idx lists; core k serves eighth k of each window.
        gi = np.zeros((128, N_WIN * idx_cols), dtype=np.int16)
        sq = src_p.reshape(N_WIN, NEIGHTH, eslots)
        for kcore in range(8):
            wrap = sq[:, kcore, :].reshape(N_WIN, idx_cols, 16)
            gi[16 * kcore:16 * (kcore + 1), :] = (
                wrap.transpose(2, 0, 1).reshape(16, N_WIN * idx_cols))

        # target one-hots: ohw[w*128+p, c*125+n], chunk c = e8*jbs + jb
        oha = (col_p[:, None] == np.arange(WIN_NODES)[None, :]).astype(np.float32)
        oha = oha.reshape(N_WIN, n_ch, 128, WIN_NODES).transpose(0, 2, 1, 3)
        oha = oha.reshape(N_WIN * 128, n_ch * WIN_NODES).astype(bf)

        # k-rel one-hot, jb-major: m4[w*128+p, jb*32 + e8*4 + f]
        m4a = (krel_p[:, None] == np.arange(4)[None, :]).astype(np.float32)
        m4a = m4a.reshape(N_WIN, NEIGHTH, jbs, 128, 4)       # [w, e, jb, p, f]
        m4a = m4a.transpose(0, 3, 2, 1, 4)                    # [w, p, jb, e, f]
        m4a = m4a.reshape(N_WIN * 128, n_ch * 4).astype(bf)

        in_maps.append({
            "featP": featP,
            "gidx": gi,
            "ohw": np.ascontiguousarray(oha),
            "m4": np.ascontiguousarray(m4a),
            "w2g": w2g,
            "ident": ident,
        })
    return in_maps


def kernel(source, target, features, hood_coords, mu, w):
    from concourse.bass_utils import run_bass_kernel_spmd

    k = _nearest_k(hood_coords, mu)
    _CACHE["k"] = k
    layout = _pick_layout(target, k)
    key = ("nc", layout)
    if key not in _CACHE:
        _CACHE[key] = _build_nc(layout)
    nc = _CACHE[key]

    in_maps = _host_prep(source, target, features, hood_coords, mu, w, layout)
    res = run_bass_kernel_spmd(nc, in_maps, list(range(NCORES)))
    _CACHE["last"] = res
    parts = [res.results[c]["out"] for c in range(NCORES)]
    return np.concatenate(parts, axis=0).astype(np.float32)
